# revision 11
# baseline (speedup 1.0000x reference)
"""Trainium2 Bass kernel for nn_AttentionBlockE3 (segment-softmax GNN attention).

Strategy: host sorts edges by destination node and partitions NODES across the
8 cores (1250 nodes each) so each core owns all edges of its nodes — no
collectives needed. Edges are packed per (core, node-chunk-of-128) into a
fixed budget of T_fix 128-edge tiles (padding edges get cutoff=0 / dst=-5 so
they contribute nothing).

v2: everything fp16 (halves the HBM traffic, which is the bottleneck), and
the per-head q.k reduction is done on the Tensor engine instead of DVE:
q and k are packed feature-transposed ([4*128 feature rows, E]); DVE forms
prod = q*k in one packed fp16 multiply (2x mode), then 4 matmuls against a
constant 0/1 head-membership matrix reduce features -> logits [128 edges, 8]
directly in PSUM. v is packed d-major (col = d*8+h) so the w*v broadcast
multiply keeps the last dim packed (DVE 2x mode); the host un-permutes the
output columns at the end.

Device program per core (all static addressing, shared by all 8 cores):
  pass 1  stream qT/kT spans, prod = qT*kT (DVE 2x), 4 membership matmuls
          -> logits psum [128,8] per tile, scale by cutoff (tensor_scalar)
  chunk   global-max-subtract softmax: exp(w - C) on ACT, C = chunk max
  pass 2  stream v tiles, rhs = wexp*v (DVE 2x d-major), one-hot via
          tensor_scalar(is_equal), matmuls accumulate [128 nodes, 480] + the
          denominator [128, 8] in PSUM
  epilog  out = pso * recip(psd + 1e-30), DMA fp16 to the node slice
"""
import numpy as np

E, D, N, H = 200000, 480, 10000, 8
P = 128
NCORES = 8
SCALE = 1.0 / np.sqrt(60.0)
FG = 4           # feature groups of 128 partitions (480 padded to 512)
SPAN = 11        # edge tiles per DMA/prod-multiply span


def _perm_dmajor():
    # packed col d*8+h  <-  fused col (per-irrep head-major layout)
    perm = np.zeros(480, np.int64)
    for h in range(8):
        for d in range(60):
            if d < 16:
                fused = h * 16 + d
            elif d < 40:
                fused = 128 + h * 24 + (d - 16)
            else:
                fused = 320 + h * 20 + (d - 40)
            perm[d * 8 + h] = fused
    return perm


PERM = _perm_dmajor()


def _memb():
    # [FG*P, 8] 0/1 head membership of each fused feature row (pad rows 0)
    m = np.zeros((FG * P, 8), np.float16)
    for f in range(480):
        if f < 128:
            h = f // 16
        elif f < 320:
            h = (f - 128) // 24
        else:
            h = (f - 320) // 20
        m[f, h] = 1.0
    return m


def _plan_shard(dst):
    npc = N // NCORES                       # 1250 nodes per core
    CHUNKS = (npc + P - 1) // P             # 10 windows of <=128 nodes
    order = np.argsort(dst, kind="stable")
    dst_s = dst[order]
    lo = np.array([core * npc + c * P
                   for core in range(NCORES) for c in range(CHUNKS)])
    hi = np.array([core * npc + min((c + 1) * P, npc)
                   for core in range(NCORES) for c in range(CHUNKS)])
    starts = np.searchsorted(dst_s, lo, side="left")
    ends = np.searchsorted(dst_s, hi, side="left")
    counts = ends - starts
    T_fix = int(np.max((counts + P - 1) // P))
    budget = T_fix * P
    gi = np.full((NCORES, CHUNKS, budget), -1, np.int64)
    for wi in range(NCORES * CHUNKS):
        core, c = wi // CHUNKS, wi % CHUNKS
        gi[core, c, :counts[wi]] = order[starts[wi]:ends[wi]]
    return gi.reshape(NCORES, -1), T_fix, CHUNKS, npc


def _pack_core(core, gi, T_fix, CHUNKS, npc, key, value, query, cutoff, dst):
    g = gi[core]
    pad = g < 0
    gc = np.clip(g, 0, E - 1)
    n = g.size
    qkT = np.zeros((2 * FG * P, n), np.float16)
    qkT[:480] = query[gc].T.astype(np.float16)
    qkT[FG * P:FG * P + 480] = key[gc].T.astype(np.float16)
    v = np.ascontiguousarray(value[gc][:, PERM].astype(np.float16))
    cut = (cutoff[gc] * SCALE).astype(np.float32)
    cut[pad] = 0.0
    chunk_of = np.repeat(np.arange(CHUNKS), T_fix * P)
    dstrel = (dst[gc] - (core * npc + chunk_of * P)).astype(np.float32)
    dstrel[pad] = -5.0
    T_tot = CHUNKS * T_fix
    cut2 = np.ascontiguousarray(cut.reshape(T_tot, P).T)
    dstrel2 = np.ascontiguousarray(dstrel.reshape(T_tot, P).T)
    return {"qkT": np.ascontiguousarray(qkT),
            "v": v, "cut": cut2, "dstr": dstrel2, "memb": _memb()}


def _build_program(T_fix, CHUNKS, reps=1):
    import contextlib

    import concourse.bacc as bacc
    import concourse.mybir as mybir
    import concourse.tile as tile
    from concourse import bass_isa

    f32 = mybir.dt.float32
    f16 = mybir.dt.float16
    T_tot = CHUNKS * T_fix
    Epc = T_tot * P

    nc = bacc.Bacc("TRN2", target_bir_lowering=False, debug=False,
                   num_devices=NCORES)
    qkT_d = nc.dram_tensor("qkT", [2 * FG * P, Epc], f16,
                           kind="ExternalInput").ap()
    v_d = nc.dram_tensor("v", [Epc, 480], f16, kind="ExternalInput").ap()
    cut_d = nc.dram_tensor("cut", [P, T_tot], f32, kind="ExternalInput").ap()
    dstr_d = nc.dram_tensor("dstr", [P, T_tot], f32, kind="ExternalInput").ap()
    memb_d = nc.dram_tensor("memb", [FG * P, 8], f16, kind="ExternalInput").ap()
    out_d = nc.dram_tensor("out", [CHUNKS * P, 480], f16,
                           kind="ExternalOutput").ap()

    with tile.TileContext(nc) as tc:
        with (
            tc.tile_pool(name="const", bufs=1) as const_pool,
            tc.tile_pool(name="qk", bufs=3) as qk_pool,
            tc.tile_pool(name="prod", bufs=3) as prod_pool,
            tc.tile_pool(name="w", bufs=4) as w_pool,
            tc.tile_pool(name="v", bufs=3) as v_pool,
            tc.tile_pool(name="rhs", bufs=4) as rhs_pool,
            tc.tile_pool(name="oh", bufs=4) as oh_pool,
            tc.tile_pool(name="stat", bufs=6) as stat_pool,
            tc.tile_pool(name="outp", bufs=3) as out_pool,
            tc.tile_pool(name="psw", bufs=3, space="PSUM") as psw_pool,
            tc.tile_pool(name="pso", bufs=2, space="PSUM") as pso_pool,
            tc.tile_pool(name="psd", bufs=2, space="PSUM") as psd_pool,
        ):
            iota_i = const_pool.tile([P, P], mybir.dt.int32)
            nc.gpsimd.iota(iota_i[:], pattern=[[1, P]], base=0,
                           channel_multiplier=0)
            iota_f = const_pool.tile([P, P], f16)
            nc.vector.tensor_copy(iota_f[:], iota_i[:])
            cut_sb = const_pool.tile([P, T_tot], f32)
            nc.sync.dma_start(out=cut_sb[:], in_=cut_d[:, :])
            dstr_sb = const_pool.tile([P, T_tot], f32)
            nc.sync.dma_start(out=dstr_sb[:], in_=dstr_d[:, :])
            memb_sb = const_pool.tile([P, FG * 8], f16)
            for g in range(FG):
                nc.sync.dma_start(out=memb_sb[:, g * 8:(g + 1) * 8],
                                  in_=memb_d[g * P:(g + 1) * P, :])

            def chunk_body(c):
                w_chunk = w_pool.tile([P, T_fix * 8], f16)
                for s0 in range(0, T_fix, SPAN):
                    sw = min(SPAN, T_fix - s0)
                    e0 = (c * T_fix + s0) * P
                    ew = sw * P
                    # one DMA for all 8 feature groups (4 of q, 4 of k)
                    qk = qk_pool.tile([P, 2 * FG * ew], f16)
                    nc.sync.dma_start(
                        out=qk[:].rearrange("p (g e) -> p g e", g=2 * FG),
                        in_=qkT_d[:, e0:e0 + ew].rearrange(
                            "(g p) e -> p g e", p=P))
                    # one fused multiply: prod[g] = q[g] * k[g] for all groups
                    pr = prod_pool.tile([P, FG * ew], f16)
                    nc.vector.tensor_mul(pr[:], qk[:, 0:FG * ew],
                                         qk[:, FG * ew:2 * FG * ew])
                    for tl in range(sw):
                        t = s0 + tl
                        gidx = c * T_fix + t
                        psw = psw_pool.tile([P, 8], f32)
                        for g in range(FG):
                            nc.tensor.matmul(
                                out=psw[:],
                                lhsT=pr[:, g * ew + tl * P:g * ew + (tl + 1) * P],
                                rhs=memb_sb[:, g * 8:(g + 1) * 8],
                                start=(g == 0), stop=(g == FG - 1))
                        nc.vector.tensor_scalar(
                            out=w_chunk[:, t * 8:(t + 1) * 8], in0=psw[:],
                            scalar1=cut_sb[:, gidx:gidx + 1], scalar2=None,
                            op0=mybir.AluOpType.mult)

                wmax = stat_pool.tile([P, 1], f32)
                nc.vector.reduce_max(out=wmax[:], in_=w_chunk[:],
                                     axis=mybir.AxisListType.X)
                cmax = stat_pool.tile([P, 1], f32)
                nc.gpsimd.partition_all_reduce(cmax[:], wmax[:], channels=P,
                                               reduce_op=bass_isa.ReduceOp.max)
                negC = stat_pool.tile([P, 1], f32)
                nc.vector.tensor_scalar_mul(negC[:], cmax[:], -1.0)
                wexp = w_pool.tile([P, T_fix * 8], f16)
                nc.scalar.activation(wexp[:], w_chunk[:],
                                     mybir.ActivationFunctionType.Exp,
                                     bias=negC[:], scale=1.0)

                pso = pso_pool.tile([P, 480], f32)
                psd = psd_pool.tile([P, 8], f32)
                for s0 in range(0, T_fix, SPAN):
                    sw = min(SPAN, T_fix - s0)
                    e0 = (c * T_fix + s0) * P
                    ew = sw * P
                    # one DMA for a span of v tiles (ACT queue: overlaps the
                    # qk stream on the SP queue)
                    vs = v_pool.tile([P, sw * 480], f16)
                    nc.scalar.dma_start(
                        out=vs[:].rearrange("p (t f) -> p t f", t=sw),
                        in_=v_d[e0:e0 + ew, :].rearrange("(t p) f -> p t f",
                                                         p=P))
                    for tl in range(sw):
                        t = s0 + tl
                        gidx = c * T_fix + t
                        vt = vs[:, tl * 480:(tl + 1) * 480]
                        rhs = rhs_pool.tile([P, 480], f16)
                        wslice = wexp[:, t * 8:(t + 1) * 8]
                        nc.vector.tensor_mul(
                            rhs[:].rearrange("p (d h) -> p d h", h=8),
                            vt.rearrange("p (d h) -> p d h", h=8),
                            wslice.unsqueeze(1).to_broadcast([P, 60, 8]))
                        oh = oh_pool.tile([P, P], f16)
                        nc.gpsimd.tensor_scalar(
                            out=oh[:], in0=iota_f[:],
                            scalar1=dstr_sb[:, gidx:gidx + 1], scalar2=None,
                            op0=mybir.AluOpType.is_equal)
                        nc.tensor.matmul(out=pso[:], lhsT=oh[:], rhs=rhs[:],
                                         start=(t == 0), stop=(t == T_fix - 1))
                        nc.tensor.matmul(out=psd[:], lhsT=oh[:], rhs=wslice,
                                         start=(t == 0), stop=(t == T_fix - 1))

                srec = stat_pool.tile([P, 8], f32)
                nc.vector.tensor_scalar_add(srec[:], psd[:], 1e-30)
                nc.vector.reciprocal(srec[:], srec[:])
                outt = out_pool.tile([P, 480], f16)
                nc.vector.tensor_mul(
                    outt[:].rearrange("p (d h) -> p d h", h=8),
                    pso[:].rearrange("p (d h) -> p d h", h=8),
                    srec[:].unsqueeze(1).to_broadcast([P, 60, 8]))
                nc.sync.dma_start(out=out_d[c * P:(c + 1) * P, :], in_=outt[:])

            # reps>1 wraps the body in a hardware loop purely for timing
            loop = tc.For_i(0, reps, 1) if reps > 1 else contextlib.nullcontext()
            with loop:
                for c in range(CHUNKS):
                    chunk_body(c)

    nc.compile()
    return nc


def _unpermute(packed):
    # packed [-, 480] d-major -> fused layout, f32
    out = np.empty((packed.shape[0], 480), np.float32)
    out[:, PERM] = packed.astype(np.float32)
    return out


def kernel(key, value, query, edge_weight_cutoff, edge_index, num_nodes):
    key = np.ascontiguousarray(np.asarray(key, dtype=np.float32))
    value = np.ascontiguousarray(np.asarray(value, dtype=np.float32))
    query = np.ascontiguousarray(np.asarray(query, dtype=np.float32))
    cutoff = np.asarray(edge_weight_cutoff, dtype=np.float32)
    dst = np.asarray(edge_index)[1].astype(np.int64)

    gi, T_fix, CHUNKS, npc = _plan_shard(dst)
    in_maps = [_pack_core(core, gi, T_fix, CHUNKS, npc,
                          key, value, query, cutoff, dst)
               for core in range(NCORES)]

    nc = _build_program(T_fix, CHUNKS)

    from concourse.bass_utils import run_bass_kernel_spmd
    res = run_bass_kernel_spmd(nc, in_maps, core_ids=list(range(NCORES)))
    out = np.concatenate([_unpermute(r["out"][:npc]) for r in res.results])
    return np.ascontiguousarray(out)


if __name__ == "__main__":
    rng = np.random.default_rng(0)
    inputs = {
        "key": rng.standard_normal((E, D)).astype(np.float32),
        "value": rng.standard_normal((E, D)).astype(np.float32),
        "query": rng.standard_normal((E, D)).astype(np.float32),
        "edge_weight_cutoff": rng.random(E).astype(np.float32),
        "edge_index": rng.integers(0, N, (2, E)),
        "num_nodes": N,
    }
    out = kernel(**inputs)
    print("out", out.shape, out.dtype, float(np.abs(out).max()))


# revision 12
# speedup vs baseline: 2.2557x; 2.2557x over previous
"""Trainium2 Bass kernel for nn_AttentionBlockE3 (segment-softmax GNN attention).

Strategy: host sorts edges by destination node and partitions NODES across the
8 cores (1250 nodes each) so each core owns all edges of its nodes — no
collectives needed. Edges are packed per (core, node-chunk-of-128) into a
fixed budget of T_fix 128-edge tiles (padding edges get cutoff=0 / dst=-5 so
they contribute nothing).

v2: everything fp16 (halves the HBM traffic, which is the bottleneck), and
the per-head q.k reduction is done on the Tensor engine instead of DVE:
q and k are packed feature-transposed ([4*128 feature rows, E]); DVE forms
prod = q*k in one packed fp16 multiply (2x mode), then 4 matmuls against a
constant 0/1 head-membership matrix reduce features -> logits [128 edges, 8]
directly in PSUM. v is packed d-major (col = d*8+h) so the w*v broadcast
multiply keeps the last dim packed (DVE 2x mode); the host un-permutes the
output columns at the end.

Device program per core (all static addressing, shared by all 8 cores):
  pass 1  stream qT/kT spans, prod = qT*kT (DVE 2x), 4 membership matmuls
          -> logits psum [128,8] per tile, scale by cutoff (tensor_scalar)
  chunk   global-max-subtract softmax: exp(w - C) on ACT, C = chunk max
  pass 2  stream v tiles, rhs = wexp*v (DVE 2x d-major), one-hot via
          tensor_scalar(is_equal), matmuls accumulate [128 nodes, 480] + the
          denominator [128, 8] in PSUM
  epilog  out = pso * recip(psd + 1e-30), DMA fp16 to the node slice
"""
import numpy as np

E, D, N, H = 200000, 480, 10000, 8
P = 128
NCORES = 8
SCALE = 1.0 / np.sqrt(60.0)
FG = 4           # feature groups of 128 partitions (480 padded to 512)
SPAN = 11        # edge tiles per DMA/prod-multiply span


def _perm_dmajor():
    # packed col d*8+h  <-  fused col (per-irrep head-major layout)
    perm = np.zeros(480, np.int64)
    for h in range(8):
        for d in range(60):
            if d < 16:
                fused = h * 16 + d
            elif d < 40:
                fused = 128 + h * 24 + (d - 16)
            else:
                fused = 320 + h * 20 + (d - 40)
            perm[d * 8 + h] = fused
    return perm


PERM = _perm_dmajor()


def _memb():
    # [FG*P, 8] 0/1 head membership of each fused feature row (pad rows 0)
    m = np.zeros((FG * P, 8), np.float16)
    for f in range(480):
        if f < 128:
            h = f // 16
        elif f < 320:
            h = (f - 128) // 24
        else:
            h = (f - 320) // 20
        m[f, h] = 1.0
    return m


def _plan_shard(dst):
    npc = N // NCORES                       # 1250 nodes per core
    CHUNKS = (npc + P - 1) // P             # 10 windows of <=128 nodes
    order = np.argsort(dst, kind="stable")
    dst_s = dst[order]
    lo = np.array([core * npc + c * P
                   for core in range(NCORES) for c in range(CHUNKS)])
    hi = np.array([core * npc + min((c + 1) * P, npc)
                   for core in range(NCORES) for c in range(CHUNKS)])
    starts = np.searchsorted(dst_s, lo, side="left")
    ends = np.searchsorted(dst_s, hi, side="left")
    counts = ends - starts
    T_fix = int(np.max((counts + P - 1) // P))
    budget = T_fix * P
    gi = np.full((NCORES, CHUNKS, budget), -1, np.int64)
    for wi in range(NCORES * CHUNKS):
        core, c = wi // CHUNKS, wi % CHUNKS
        gi[core, c, :counts[wi]] = order[starts[wi]:ends[wi]]
    return gi.reshape(NCORES, -1), T_fix, CHUNKS, npc


def _pack_core(core, gi, T_fix, CHUNKS, npc, key, value, query, cutoff, dst):
    g = gi[core]
    pad = g < 0
    gc = np.clip(g, 0, E - 1)
    n = g.size
    qkT = np.zeros((2 * FG * P, n), np.float16)
    qkT[:480] = query[gc].T.astype(np.float16)
    qkT[FG * P:FG * P + 480] = key[gc].T.astype(np.float16)
    v = np.ascontiguousarray(value[gc][:, PERM].astype(np.float16))
    cut = (cutoff[gc] * SCALE).astype(np.float32)
    cut[pad] = 0.0
    chunk_of = np.repeat(np.arange(CHUNKS), T_fix * P)
    dstrel = (dst[gc] - (core * npc + chunk_of * P)).astype(np.float32)
    dstrel[pad] = -5.0
    T_tot = CHUNKS * T_fix
    cut2 = np.ascontiguousarray(cut.reshape(T_tot, P).T)
    dstrel2 = np.ascontiguousarray(dstrel.reshape(T_tot, P).T)
    return {"qkT": np.ascontiguousarray(qkT),
            "v": v, "cut": cut2, "dstr": dstrel2, "memb": _memb()}


def _build_program(T_fix, CHUNKS, reps=1):
    import contextlib

    import concourse.bacc as bacc
    import concourse.mybir as mybir
    import concourse.tile as tile
    from concourse import bass_isa

    f32 = mybir.dt.float32
    f16 = mybir.dt.float16
    T_tot = CHUNKS * T_fix
    Epc = T_tot * P

    nc = bacc.Bacc("TRN2", target_bir_lowering=False, debug=False,
                   num_devices=NCORES)
    qkT_d = nc.dram_tensor("qkT", [2 * FG * P, Epc], f16,
                           kind="ExternalInput").ap()
    v_d = nc.dram_tensor("v", [Epc, 480], f16, kind="ExternalInput").ap()
    cut_d = nc.dram_tensor("cut", [P, T_tot], f32, kind="ExternalInput").ap()
    dstr_d = nc.dram_tensor("dstr", [P, T_tot], f32, kind="ExternalInput").ap()
    memb_d = nc.dram_tensor("memb", [FG * P, 8], f16, kind="ExternalInput").ap()
    out_d = nc.dram_tensor("out", [CHUNKS * P, 480], f16,
                           kind="ExternalOutput").ap()

    with tile.TileContext(nc) as tc:
        with (
            tc.tile_pool(name="const", bufs=1) as const_pool,
            tc.tile_pool(name="qk", bufs=3) as qk_pool,
            tc.tile_pool(name="prod", bufs=3) as prod_pool,
            tc.tile_pool(name="w", bufs=4) as w_pool,
            tc.tile_pool(name="v", bufs=3) as v_pool,
            tc.tile_pool(name="rhs", bufs=4) as rhs_pool,
            tc.tile_pool(name="oh", bufs=4) as oh_pool,
            tc.tile_pool(name="stat", bufs=6) as stat_pool,
            tc.tile_pool(name="outp", bufs=3) as out_pool,
            tc.tile_pool(name="psw", bufs=3, space="PSUM") as psw_pool,
            tc.tile_pool(name="pso", bufs=2, space="PSUM") as pso_pool,
            tc.tile_pool(name="psd", bufs=2, space="PSUM") as psd_pool,
        ):
            iota_i = const_pool.tile([P, P], mybir.dt.int32)
            nc.gpsimd.iota(iota_i[:], pattern=[[1, P]], base=0,
                           channel_multiplier=0)
            iota_f = const_pool.tile([P, P], f16)
            nc.vector.tensor_copy(iota_f[:], iota_i[:])
            cut_sb = const_pool.tile([P, T_tot], f32)
            nc.sync.dma_start(out=cut_sb[:], in_=cut_d[:, :])
            dstr_sb = const_pool.tile([P, T_tot], f32)
            nc.sync.dma_start(out=dstr_sb[:], in_=dstr_d[:, :])
            memb_sb = const_pool.tile([P, FG * 8], f16)
            for g in range(FG):
                nc.sync.dma_start(out=memb_sb[:, g * 8:(g + 1) * 8],
                                  in_=memb_d[g * P:(g + 1) * P, :])

            def chunk_body(c):
                w_chunk = w_pool.tile([P, T_fix * 8], f16)
                for s0 in range(0, T_fix, SPAN):
                    sw = min(SPAN, T_fix - s0)
                    e0 = (c * T_fix + s0) * P
                    ew = sw * P
                    # one DMA for all 8 feature groups (4 of q, 4 of k)
                    qk = qk_pool.tile([P, 2 * FG * ew], f16)
                    nc.sync.dma_start(
                        out=qk[:].rearrange("p (g e) -> p g e", g=2 * FG),
                        in_=qkT_d[:, e0:e0 + ew].rearrange(
                            "(g p) e -> p g e", p=P))
                    # one fused multiply: prod[g] = q[g] * k[g] for all groups
                    pr = prod_pool.tile([P, FG * ew], f16)
                    nc.vector.tensor_mul(pr[:], qk[:, 0:FG * ew],
                                         qk[:, FG * ew:2 * FG * ew])
                    for tl in range(sw):
                        t = s0 + tl
                        gidx = c * T_fix + t
                        psw = psw_pool.tile([P, 8], f32)
                        for g in range(FG):
                            nc.tensor.matmul(
                                out=psw[:],
                                lhsT=pr[:, g * ew + tl * P:g * ew + (tl + 1) * P],
                                rhs=memb_sb[:, g * 8:(g + 1) * 8],
                                start=(g == 0), stop=(g == FG - 1))
                        nc.vector.tensor_scalar(
                            out=w_chunk[:, t * 8:(t + 1) * 8], in0=psw[:],
                            scalar1=cut_sb[:, gidx:gidx + 1], scalar2=None,
                            op0=mybir.AluOpType.mult)

                wmax = stat_pool.tile([P, 1], f32)
                nc.vector.reduce_max(out=wmax[:], in_=w_chunk[:],
                                     axis=mybir.AxisListType.X)
                cmax = stat_pool.tile([P, 1], f32)
                nc.gpsimd.partition_all_reduce(cmax[:], wmax[:], channels=P,
                                               reduce_op=bass_isa.ReduceOp.max)
                negC = stat_pool.tile([P, 1], f32)
                nc.vector.tensor_scalar_mul(negC[:], cmax[:], -1.0)
                wexp = w_pool.tile([P, T_fix * 8], f16)
                nc.scalar.activation(wexp[:], w_chunk[:],
                                     mybir.ActivationFunctionType.Exp,
                                     bias=negC[:], scale=1.0)

                pso = pso_pool.tile([P, 480], f32)
                psd = psd_pool.tile([P, 8], f32)
                for s0 in range(0, T_fix, SPAN):
                    sw = min(SPAN, T_fix - s0)
                    e0 = (c * T_fix + s0) * P
                    ew = sw * P
                    # one DMA for a span of v tiles (ACT queue: overlaps the
                    # qk stream on the SP queue)
                    vs = v_pool.tile([P, sw * 480], f16)
                    nc.scalar.dma_start(
                        out=vs[:].rearrange("p (t f) -> p t f", t=sw),
                        in_=v_d[e0:e0 + ew, :].rearrange("(t p) f -> p t f",
                                                         p=P))
                    for tl in range(sw):
                        t = s0 + tl
                        gidx = c * T_fix + t
                        vt = vs[:, tl * 480:(tl + 1) * 480]
                        rhs = rhs_pool.tile([P, 480], f16)
                        wslice = wexp[:, t * 8:(t + 1) * 8]
                        nc.vector.tensor_mul(
                            rhs[:].rearrange("p (d h) -> p d h", h=8),
                            vt.rearrange("p (d h) -> p d h", h=8),
                            wslice.unsqueeze(1).to_broadcast([P, 60, 8]))
                        oh = oh_pool.tile([P, P], f16)
                        nc.vector.tensor_scalar(
                            out=oh[:], in0=iota_f[:],
                            scalar1=dstr_sb[:, gidx:gidx + 1], scalar2=None,
                            op0=mybir.AluOpType.is_equal)
                        nc.tensor.matmul(out=pso[:], lhsT=oh[:], rhs=rhs[:],
                                         start=(t == 0), stop=(t == T_fix - 1))
                        nc.tensor.matmul(out=psd[:], lhsT=oh[:], rhs=wslice,
                                         start=(t == 0), stop=(t == T_fix - 1))

                srec = stat_pool.tile([P, 8], f32)
                nc.vector.tensor_scalar_add(srec[:], psd[:], 1e-30)
                nc.vector.reciprocal(srec[:], srec[:])
                outt = out_pool.tile([P, 480], f16)
                nc.vector.tensor_mul(
                    outt[:].rearrange("p (d h) -> p d h", h=8),
                    pso[:].rearrange("p (d h) -> p d h", h=8),
                    srec[:].unsqueeze(1).to_broadcast([P, 60, 8]))
                nc.sync.dma_start(out=out_d[c * P:(c + 1) * P, :], in_=outt[:])

            # reps>1 wraps the body in a hardware loop purely for timing
            loop = tc.For_i(0, reps, 1) if reps > 1 else contextlib.nullcontext()
            with loop:
                for c in range(CHUNKS):
                    chunk_body(c)

    nc.compile()
    return nc


def _unpermute(packed):
    # packed [-, 480] d-major -> fused layout, f32
    out = np.empty((packed.shape[0], 480), np.float32)
    out[:, PERM] = packed.astype(np.float32)
    return out


def kernel(key, value, query, edge_weight_cutoff, edge_index, num_nodes):
    key = np.ascontiguousarray(np.asarray(key, dtype=np.float32))
    value = np.ascontiguousarray(np.asarray(value, dtype=np.float32))
    query = np.ascontiguousarray(np.asarray(query, dtype=np.float32))
    cutoff = np.asarray(edge_weight_cutoff, dtype=np.float32)
    dst = np.asarray(edge_index)[1].astype(np.int64)

    gi, T_fix, CHUNKS, npc = _plan_shard(dst)
    in_maps = [_pack_core(core, gi, T_fix, CHUNKS, npc,
                          key, value, query, cutoff, dst)
               for core in range(NCORES)]

    nc = _build_program(T_fix, CHUNKS)

    from concourse.bass_utils import run_bass_kernel_spmd
    res = run_bass_kernel_spmd(nc, in_maps, core_ids=list(range(NCORES)))
    out = np.concatenate([_unpermute(r["out"][:npc]) for r in res.results])
    return np.ascontiguousarray(out)


if __name__ == "__main__":
    rng = np.random.default_rng(0)
    inputs = {
        "key": rng.standard_normal((E, D)).astype(np.float32),
        "value": rng.standard_normal((E, D)).astype(np.float32),
        "query": rng.standard_normal((E, D)).astype(np.float32),
        "edge_weight_cutoff": rng.random(E).astype(np.float32),
        "edge_index": rng.integers(0, N, (2, E)),
        "num_nodes": N,
    }
    out = kernel(**inputs)
    print("out", out.shape, out.dtype, float(np.abs(out).max()))
